# revision 1
# baseline (speedup 1.0000x reference)
"""Causal self-attention (B=4, T=2048, C=1024, H=16) on 8 TRN2 NeuronCores.

Sharding: data-parallel over B (4) x tensor-parallel over heads (2 halves of 8
heads). Core c handles batch c//2, heads 8*(c%2) .. 8*(c%2)+8. Each core runs
the full pipeline for its (batch, head-half): QKV projection, causal
attention, output projection against its 512 rows of w_proj, producing a
partial [C, T] output; the host sums core pairs and transposes.

Attention is computed in the S^T = K^T-major layout so no on-chip transposes
are needed: softmax denominators come from a ones-column appended to V, and
the division is broadcast across partitions with a rank-1 PE matmul.

All matmuls run in fp16 (fp32 PSUM accumulation). fp16 keeps 10 mantissa
bits; every tensor in this problem is O(1)-ranged so there is no overflow
risk, and measured end-to-end error vs the fp32 reference is ~1e-3.
"""

import sys

if "/opt/trn_rl_repo" not in sys.path:
    sys.path.insert(0, "/opt/trn_rl_repo")

from contextlib import ExitStack

import numpy as np

import concourse.tile as tile
from concourse import bacc, mybir

F32 = mybir.dt.float32
F32R = mybir.dt.float32r
FP16 = mybir.dt.float16

B, T, C, H = 4, 2048, 1024, 16
HL = 8  # heads per core
HD = 64  # head dim
CL = HL * HD  # local channel width (512)
NT = T // 512  # 4 t-chunks of 512
NK = C // 128  # 8 contraction tiles over C
NTT = T // 128  # 16 t1/t2 tiles of 128


def build_nc(split_moving=False):
    nc = bacc.Bacc(None)

    xT_d = nc.declare_dram_parameter("xT", [C, T], FP16, isOutput=False)
    wqk_d = nc.declare_dram_parameter("wqk", [C, 2 * CL], FP16, isOutput=False)
    wv_d = nc.declare_dram_parameter("wv", [C, CL], FP16, isOutput=False)
    wproj_d = nc.declare_dram_parameter("wproj", [HL, HD, C], FP16, isOutput=False)
    bqk_d = nc.declare_dram_parameter("bqk", [128, 8], F32, isOutput=False)
    bv_d = nc.declare_dram_parameter("bv", [128, CL], F32, isOutput=False)
    bproj_d = nc.declare_dram_parameter("bproj", [128, 8], F32, isOutput=False)
    outT_d = nc.declare_dram_parameter("outT", [C, T], F32, isOutput=True)

    with tile.TileContext(nc) as tc, ExitStack() as ctx:
        # ---------- persistent pools ----------
        persist = ctx.enter_context(tc.tile_pool(name="persist", bufs=1))
        qkT = []  # 8 tiles [128, T] fp16: rows = qkv-column block nn
        for nn in range(8):
            t_qk = persist.tile([128, T], FP16, tag=f"qkT{nn}")
            qkT.append(t_qk)
        vaug = []  # 16 tiles [128, 8*65] fp16: v (64 cols) + ones col per head
        for j in range(NTT):
            t_va = persist.tile([128, HL * 65], FP16, tag=f"vaug{j}")
            vaug.append(t_va)
        bqk_sb = persist.tile([128, 8], F32, tag="bqk")
        bv_sb = persist.tile([128, CL], F32, tag="bv")
        bproj_sb = persist.tile([128, 8], F32, tag="bproj")
        ones_f32 = persist.tile([128, HD], F32, tag="ones_f32")
        ones_sb = persist.tile([128, HD], F32R, tag="ones")
        wproj_sb = []  # per head [64, C]
        for h in range(HL):
            t_wp = persist.tile([64, C], FP16, tag=f"wproj{h}")
            wproj_sb.append(t_wp)

        nc.sync.dma_start(bqk_sb[:], bqk_d[:])
        nc.sync.dma_start(bv_sb[:], bv_d[:])
        nc.sync.dma_start(bproj_sb[:], bproj_d[:])
        nc.vector.memset(ones_f32[:], 1.0)
        nc.vector.tensor_copy(ones_sb[:], ones_f32[:])
        for h in range(HL):
            nc.sync.dma_start(wproj_sb[h][:], wproj_d[h])
        for j in range(NTT):
            # ones columns (64::65) for the PV row-sum trick
            nc.vector.memset(vaug[j][:, 64 :: 65], 1.0)

        # y storage: per head [64, T] fp16 (base partition 0 so DVE ops stay
        # partition-aligned with the [65, 512] PSUM accumulators)
        yT_sb = []
        for h in range(HL):
            t_y = persist.tile([64, T], FP16, tag=f"yT{h}")
            yT_sb.append(t_y)

        with (
            tc.tile_pool(name="wqks", bufs=1) as wqks,
            tc.tile_pool(name="xts", bufs=1) as xts,
            tc.tile_pool(name="pts", bufs=6) as pts,
            tc.tile_pool(name="rts", bufs=3) as rts,
            tc.tile_pool(name="ots", bufs=3) as ots,
            tc.tile_pool(name="pp", bufs=2, space="PSUM") as pp,
            tc.tile_pool(name="ps_y", bufs=2, space="PSUM") as ps_y,
            tc.tile_pool(name="ps_b", bufs=1, space="PSUM") as ps_b,
            tc.tile_pool(name="ps_o", bufs=1, space="PSUM") as ps_o,
        ):
            # resident x^T (ordered so chunk 0 lands first) and weights
            wv_sb = []
            for kc in range(NK):
                t_wv = wqks.tile([128, CL], FP16, tag=f"wv{kc}")
                nc.sync.dma_start(t_wv[:], wv_d[kc * 128 : (kc + 1) * 128, :])
                wv_sb.append(t_wv)
            xt = [[None] * NK for _ in range(NT)]
            for tc_ in range(NT):
                for kc in range(NK):
                    t_x = xts.tile([128, 512], FP16, tag=f"xt{tc_}_{kc}")
                    nc.sync.dma_start(
                        t_x[:],
                        xT_d[kc * 128 : (kc + 1) * 128, tc_ * 512 : (tc_ + 1) * 512],
                    )
                    xt[tc_][kc] = t_x
            wqk_sb = []
            for kc in range(NK):
                t_wqk = wqks.tile([128, 2 * CL], FP16, tag=f"wqk{kc}")
                nc.sync.dma_start(t_wqk[:], wqk_d[kc * 128 : (kc + 1) * 128, :])
                wqk_sb.append(t_wqk)

            # ---- v first: unlocks PV for every head ----
            for tc_ in range(NT):
                for ti in range(4):
                    j = tc_ * 4 + ti
                    p_v = pp.tile([128, 1024], F32, tag="pp")
                    for kc in range(NK):
                        nc.tensor.matmul(
                            p_v[:, 0:512],
                            xt[tc_][kc][:, ti * 128 : (ti + 1) * 128],
                            wv_sb[kc][:],
                            start=(kc == 0),
                            stop=(kc == NK - 1),
                        )
                    nc.vector.tensor_tensor(
                        vaug[j][:].rearrange("p (h c) -> p h c", h=HL)[:, :, 0:64],
                        p_v[:, 0:512].rearrange("p (h c) -> p h c", h=HL),
                        bv_sb[:].rearrange("p (h c) -> p h c", h=HL),
                        mybir.AluOpType.add,
                    )

            def qk_block(nn):
                # q^T (or k^T) block nn over all t, paired t-chunks per psum
                for tcp in range(2):
                    p_qk = pp.tile([128, 1024], F32, tag="pp")
                    for half in range(2):
                        tc_ = 2 * tcp + half
                        for kc in range(NK):
                            nc.tensor.matmul(
                                p_qk[:, half * 512 : half * 512 + 512],
                                wqk_sb[kc][:, nn * 128 : (nn + 1) * 128],
                                xt[tc_][kc][:],
                                start=(kc == 0),
                                stop=(kc == NK - 1),
                            )
                    nc.vector.tensor_scalar_add(
                        qkT[nn][:, tcp * 1024 : (tcp + 1) * 1024],
                        p_qk[:],
                        bqk_sb[:, nn : nn + 1],
                    )

            def attention(h):
                poff = (h % 2) * 64
                kt = qkT[4 + h // 2]
                qt = qkT[h // 2]
                for c in range(NT):
                    p_y = ps_y.tile([65, 512], F32, tag="py")
                    nj = 4 * c + 4
                    for j0 in range(0, nj, 2):
                        p_s = pp.tile([128, 1024], F32, tag="pp")
                        for half in range(2):
                            j = j0 + half
                            nc.tensor.matmul(
                                p_s[:, half * 512 : half * 512 + 512],
                                kt[poff : poff + 64, j * 128 : (j + 1) * 128],
                                qt[poff : poff + 64, c * 512 : (c + 1) * 512],
                                start=True,
                                stop=True,
                            )
                        pt = pts.tile([128, 1024], FP16, tag="pt")
                        nc.scalar.activation(
                            pt[:], p_s[:], mybir.ActivationFunctionType.Exp
                        )
                        if j0 // 4 == c:
                            # zero strictly-upper entries of both halves:
                            # keep where 512c - 128(j0+half) + f - p >= 0
                            ptm = pts.tile([128, 1024], FP16, tag="ptm")
                            nc.gpsimd.affine_select(
                                ptm[:].rearrange("p (s f) -> p s f", s=2),
                                pt[:].rearrange("p (s f) -> p s f", s=2),
                                pattern=[[-128, 2], [1, 512]],
                                compare_op=mybir.AluOpType.is_ge,
                                fill=0.0,
                                base=512 * c - 128 * j0,
                                channel_multiplier=-1,
                            )
                            pt = ptm
                        for half in range(2):
                            j = j0 + half
                            nc.tensor.matmul(
                                p_y[:],
                                vaug[j][:, h * 65 : (h + 1) * 65],
                                pt[:, half * 512 : half * 512 + 512],
                                start=(j == 0),
                                stop=(j == nj - 1),
                            )
                    # normalize: y[d, t] = y_aug[d, t] / y_aug[64, t]
                    r_sb = rts.tile([128, 512], F32R, tag="r")
                    with nc.allow_low_precision(
                        reason="f32r holds full fp32-rounded reciprocal"
                    ):
                        nc.vector.reciprocal(r_sb[64:65, :], p_y[64:65, :])
                    p_r = ps_b.tile([64, 512], F32, tag="pb")
                    nc.tensor.matmul(
                        p_r[:],
                        ones_sb[64:65, :],
                        r_sb[64:65, :],
                        start=True,
                        stop=True,
                    )
                    rb_sb = rts.tile([64, 512], F32, tag="rb")
                    nc.vector.tensor_copy(rb_sb[:], p_r[:])
                    nc.vector.tensor_mul(
                        yT_sb[h][:, c * 512 : (c + 1) * 512], p_y[0:64, :], rb_sb[:]
                    )

            # interleave: each qk n-tile pair unlocks two heads of attention
            for i in range(4):
                qk_block(i)
                qk_block(4 + i)
                attention(2 * i)
                attention(2 * i + 1)

            # ---- proj ----
            for c in range(NT):
                for co in range(8):
                    p_o = ps_o.tile([128, 512], F32, tag="po")
                    for h in range(HL):
                        nc.tensor.matmul(
                            p_o[:],
                            wproj_sb[h][:, co * 128 : (co + 1) * 128],
                            yT_sb[h][:, c * 512 : (c + 1) * 512],
                            start=(h == 0),
                            stop=(h == HL - 1),
                        )
                    o_sb = ots.tile([128, 512], F32, tag="o")
                    nc.vector.tensor_scalar_add(
                        o_sb[:], p_o[:], bproj_sb[:, co : co + 1]
                    )
                    nc.sync.dma_start(
                        outT_d[co * 128 : (co + 1) * 128, c * 512 : (c + 1) * 512],
                        o_sb[:],
                    )

    nc.compile()
    return nc


# ---------------------------------------------------------------------------
# host side
# ---------------------------------------------------------------------------

_CACHE = {}


def _get_runner():
    if "runner" in _CACHE:
        return _CACHE["runner"]

    import jax
    from jax.experimental.shard_map import shard_map
    from jax.sharding import Mesh, PartitionSpec

    from concourse.bass2jax import (
        _bass_exec_p,
        install_neuronx_cc_hook,
        partition_id_tensor,
    )

    install_neuronx_cc_hook()
    nc = build_nc()
    n_cores = 8

    partition_name = nc.partition_id_tensor.name if nc.partition_id_tensor else None
    in_names = []
    out_names = []
    out_avals = []
    for alloc in nc.m.functions[0].allocations:
        if not isinstance(alloc, mybir.MemoryLocationSet):
            continue
        name = alloc.memorylocations[0].name
        if alloc.kind == "ExternalInput":
            if name != partition_name:
                in_names.append(name)
        elif alloc.kind == "ExternalOutput":
            out_names.append(name)
            out_avals.append(
                jax.core.ShapedArray(tuple(alloc.tensor_shape), mybir.dt.np(alloc.dtype))
            )
    n_params = len(in_names)
    all_names = in_names + out_names
    if partition_name is not None:
        all_names = all_names + [partition_name]

    def _body(*args):
        operands = list(args)
        if partition_name is not None:
            operands.append(partition_id_tensor())
        outs = _bass_exec_p.bind(
            *operands,
            out_avals=tuple(out_avals),
            in_names=tuple(all_names),
            out_names=tuple(out_names),
            lowering_input_output_aliases=(),
            sim_require_finite=True,
            sim_require_nnan=True,
            nc=nc,
        )
        return tuple(outs)

    devices = jax.devices()[:n_cores]
    mesh = Mesh(np.asarray(devices), ("core",))
    n_outs = len(out_names)
    fn = jax.jit(
        shard_map(
            _body,
            mesh=mesh,
            in_specs=(PartitionSpec("core"),) * (n_params + n_outs),
            out_specs=(PartitionSpec("core"),) * n_outs,
            check_rep=False,
        ),
        keep_unused=True,
    )

    runner = {
        "fn": fn,
        "in_names": in_names,
        "out_names": out_names,
        "out_avals": out_avals,
        "n_cores": n_cores,
        "jax": jax,
    }
    _CACHE["runner"] = runner
    return runner


def _prepare_in_maps(x, w_attn, b_attn, w_proj, b_proj):
    x = np.asarray(x, dtype=np.float32)
    w_attn = np.asarray(w_attn, dtype=np.float32)
    b_attn = np.asarray(b_attn, dtype=np.float32)
    w_proj = np.asarray(w_proj, dtype=np.float32)
    b_proj = np.asarray(b_proj, dtype=np.float32)

    in_maps = []
    for core in range(8):
        b = core // 2
        h0 = HL * (core % 2)
        c0 = h0 * HD  # 512*(core%2)

        xT = np.ascontiguousarray(x[b].T).astype(np.float16)

        w_q = (w_attn[:, c0 : c0 + CL] * 0.125).astype(np.float16)
        w_k = w_attn[:, C + c0 : C + c0 + CL].astype(np.float16)
        wqk = np.ascontiguousarray(np.concatenate([w_q, w_k], axis=1))
        wv = np.ascontiguousarray(w_attn[:, 2 * C + c0 : 2 * C + c0 + CL]).astype(
            np.float16
        )
        wproj = np.ascontiguousarray(
            w_proj[c0 : c0 + CL, :].reshape(HL, HD, C)
        ).astype(np.float16)

        b_q = b_attn[c0 : c0 + CL] * 0.125
        b_k = b_attn[C + c0 : C + c0 + CL]
        bqk = np.concatenate([b_q, b_k]).reshape(8, 128).T.astype(np.float32)
        bqk = np.ascontiguousarray(bqk)
        b_v = b_attn[2 * C + c0 : 2 * C + c0 + CL].astype(np.float32)
        bv = np.ascontiguousarray(np.broadcast_to(b_v[None, :], (128, CL)))
        if core % 2 == 0:
            bp = np.ascontiguousarray(b_proj.reshape(8, 128).T.astype(np.float32))
        else:
            bp = np.zeros((128, 8), dtype=np.float32)

        in_maps.append(
            {
                "xT": xT,
                "wqk": wqk,
                "wv": wv,
                "wproj": wproj,
                "bqk": bqk,
                "bv": bv,
                "bproj": bp,
            }
        )
    return in_maps


def _run_device(in_maps):
    r = _get_runner()
    jax = r["jax"]
    n = r["n_cores"]
    per_core = [[np.asarray(m[name]) for name in r["in_names"]] for m in in_maps]
    concat_in = [
        np.concatenate([per_core[c][i] for c in range(n)], axis=0)
        for i in range(len(r["in_names"]))
    ]
    concat_zero = [
        np.zeros((n * a.shape[0], *a.shape[1:]), a.dtype) for a in r["out_avals"]
    ]
    outs = r["fn"](*[jax.device_put(a) for a in concat_in + concat_zero])
    jax.block_until_ready(outs)
    (outT,) = [np.asarray(o) for o in outs]
    return outT.reshape(n, C, T)


def kernel(x, w_attn, b_attn, w_proj, b_proj):
    in_maps = _prepare_in_maps(x, w_attn, b_attn, w_proj, b_proj)
    outT = _run_device(in_maps)
    # host gather: sum the two head-halves of each batch, transpose back
    out = np.empty((B, T, C), dtype=np.float32)
    for b in range(B):
        out[b] = (outT[2 * b] + outT[2 * b + 1]).T
    return out



# revision 16
# speedup vs baseline: 1.4873x; 1.4873x over previous
"""Causal self-attention (B=4, T=2048, C=1024, H=16) on 8 TRN2 NeuronCores.

Sharding: data-parallel over B (4) x tensor-parallel over heads (2 halves of 8
heads). Core c handles batch c//2, heads 8*(c%2) .. 8*(c%2)+8. Each core runs
the full pipeline for its (batch, head-half): QKV projection, causal
attention, output projection against its 512 rows of w_proj, producing a
partial [C, T] output; the host sums core pairs and transposes.

Attention is computed in the S^T = K^T-major layout so no on-chip transposes
are needed: softmax denominators come from a ones-column appended to V, and
the division is broadcast across partitions with a rank-1 PE matmul.

The causal diagonal 512x512 block is processed at 128-key-tile granularity:
only the valid (k <= q) query range of each diagonal key-tile is computed
(N = 512/384/256/128), the 128x128 triangular corners are masked post-exp
with affine_select, and PV accumulates the restricted column ranges. This
trims ~25% of attention matmul rows and softmax traffic.

y is stored as four [128, T] head-pair tiles (odd heads written at partition
offset 64) so the output projection contracts K=128 per matmul - 4 instead of
8 accumulation matmuls per output tile.

All matmuls run in fp16 (fp32 PSUM accumulation). fp16 keeps 10 mantissa
bits; every tensor in this problem is O(1)-ranged so there is no overflow
risk, and measured end-to-end error vs the fp32 reference is ~1e-3.
"""

import sys

if "/opt/trn_rl_repo" not in sys.path:
    sys.path.insert(0, "/opt/trn_rl_repo")

from contextlib import ExitStack

import numpy as np

import concourse.tile as tile
from concourse import bacc, mybir

F32 = mybir.dt.float32
F32R = mybir.dt.float32r
FP16 = mybir.dt.float16

B, T, C, H = 4, 2048, 1024, 16
HL = 8  # heads per core
HD = 64  # head dim
CL = HL * HD  # local channel width (512)
NT = T // 512  # 4 t-chunks of 512
NK = C // 128  # 8 contraction tiles over C
NTT = T // 128  # 16 t1/t2 tiles of 128


def build_nc(split_moving=False):
    nc = bacc.Bacc(None)

    xT_d = nc.declare_dram_parameter("xT", [C, T], FP16, isOutput=False)
    wqk_d = nc.declare_dram_parameter("wqk", [C, 2 * CL], FP16, isOutput=False)
    wv_d = nc.declare_dram_parameter("wv", [C, CL], FP16, isOutput=False)
    wproj_d = nc.declare_dram_parameter("wproj", [4, 128, C], FP16, isOutput=False)
    bqk_d = nc.declare_dram_parameter("bqk", [128, 8], F32, isOutput=False)
    bv_d = nc.declare_dram_parameter("bv", [128, CL], F32, isOutput=False)
    bproj_d = nc.declare_dram_parameter("bproj", [128, 8], F32, isOutput=False)
    outT_d = nc.declare_dram_parameter("outT", [C, T], F32, isOutput=True)

    with tile.TileContext(nc) as tc, ExitStack() as ctx:
        # ---------- persistent pools ----------
        persist = ctx.enter_context(tc.tile_pool(name="persist", bufs=1))
        qkT = []  # 8 tiles [128, T] fp16: rows = qkv-column block nn
        for nn in range(8):
            t_qk = persist.tile([128, T], FP16, tag=f"qkT{nn}")
            qkT.append(t_qk)
        vaug = []  # 16 tiles [128, 8*65] fp16: v (64 cols) + ones col per head
        for j in range(NTT):
            t_va = persist.tile([128, HL * 65], FP16, tag=f"vaug{j}")
            vaug.append(t_va)
        ones_f32 = persist.tile([128, HD], F32, tag="ones_f32")
        ones_sb = persist.tile([128, HD], F32R, tag="ones")
        bqk_sb = persist.tile([128, 8], F32, tag="bqk")
        bv_sb = persist.tile([128, CL], F32, tag="bv")
        bproj_sb = persist.tile([128, 8], F32, tag="bproj")
        wproj_all = persist.tile([128, 4 * C], FP16, tag="wproj")
        wproj_sb = [wproj_all[:, hh * C : (hh + 1) * C] for hh in range(4)]

        nc.vector.memset(ones_f32[:], 1.0)
        nc.vector.tensor_copy(ones_sb[:], ones_f32[:])
        for j in range(NTT):
            # ones columns (64::65) for the PV row-sum trick
            nc.vector.memset(vaug[j][:, 64 :: 65], 1.0)

        # y storage: per head-pair [128, T] fp16 (head 2*hh in partitions
        # 0-63, head 2*hh+1 in partitions 64-127) so proj contracts K=128
        yT_sb = []
        for hh in range(4):
            t_y = persist.tile([128, T], FP16, tag=f"yT{hh}")
            yT_sb.append(t_y)

        with (
            tc.tile_pool(name="wqks", bufs=1) as wqks,
            tc.tile_pool(name="xts", bufs=1) as xts,
            tc.tile_pool(name="pts", bufs=6) as pts,
            tc.tile_pool(name="tri", bufs=4) as tri,
            tc.tile_pool(name="rts", bufs=3) as rts,
            tc.tile_pool(name="ots", bufs=3) as ots,
            tc.tile_pool(name="pp", bufs=2, space="PSUM") as pp,
            tc.tile_pool(name="ps_y", bufs=2, space="PSUM") as ps_y,
            tc.tile_pool(name="ps_f", bufs=1, space="PSUM") as ps_f,
            tc.tile_pool(name="ps_o", bufs=1, space="PSUM") as ps_o,
        ):
            # resident x^T and weights, packed so each load is ONE wide DMA
            # (issue serialization on the sync sequencer is ~625ns/DMA).
            # Order = need order: wv + x chunk 0 gate the first v block, wqk
            # the first qk block; wproj is only needed at proj time.
            wv_all = wqks.tile([128, NK * CL], FP16, tag="wv")
            wv_sb = [wv_all[:, kc * CL : (kc + 1) * CL] for kc in range(NK)]
            xt_all = [None] * NT
            xt = [[None] * NK for _ in range(NT)]

            def load_x(tc_, half=None):
                if xt_all[tc_] is None:
                    t_x = xts.tile(
                        [128, NK * 512], FP16, tag=f"xt{tc_}", name="t_x"
                    )
                    xt_all[tc_] = t_x
                    for kc in range(NK):
                        xt[tc_][kc] = t_x[:, kc * 512 : (kc + 1) * 512]
                t_x = xt_all[tc_]
                ks = range(NK) if half is None else range(half * 4, half * 4 + 4)
                k0 = ks[0]
                kn = len(ks)
                nc.sync.dma_start(
                    t_x[:, k0 * 512 : (k0 + kn) * 512].rearrange(
                        "p (k f) -> p k f", k=kn
                    ),
                    xT_d[
                        k0 * 128 : (k0 + kn) * 128,
                        tc_ * 512 : (tc_ + 1) * 512,
                    ].rearrange("(k p) f -> p k f", p=128),
                )

            def load_wv(half):
                nc.sync.dma_start(
                    wv_all[:, half * 4 * CL : (half + 1) * 4 * CL].rearrange(
                        "p (k f) -> p k f", k=4
                    ),
                    wv_d[half * 512 : (half + 1) * 512, :].rearrange(
                        "(k p) f -> p k f", p=128
                    ),
                )

            load_wv(0)
            load_x(0, 0)
            load_wv(1)
            load_x(0, 1)
            nc.sync.dma_start(bqk_sb[:], bqk_d[:])
            nc.sync.dma_start(bv_sb[:], bv_d[:])
            wqk_all = wqks.tile([128, NK * 2 * CL], FP16, tag="wqk")
            nc.sync.dma_start(
                wqk_all[:].rearrange("p (k f) -> p k f", k=NK),
                wqk_d[:].rearrange("(k p) f -> p k f", p=128),
            )
            wqk_sb = [
                wqk_all[:, kc * 2 * CL : (kc + 1) * 2 * CL] for kc in range(NK)
            ]
            load_x(1)
            load_x(2)
            load_x(3)
            nc.sync.dma_start(bproj_sb[:], bproj_d[:])
            nc.sync.dma_start(
                wproj_all[:].rearrange("p (k f) -> p k f", k=4),
                wproj_d[:].rearrange("k p f -> p k f"),
            )

            # ---- filler feeder: weaves QKV-projection / output-projection
            # matmul units between attention pairs so the in-order PE queue
            # never stalls on exp latency. Units are ~2 matmuls (~430ns).
            from collections import deque

            finish_pending = []

            fq = deque()  # generators
            fcur = [None]
            fpend = [0]  # approximate pending feed-units

            def feeder_add(gen, units):
                fq.append(gen)
                fpend[0] += units

            def feed(n=1):
                for _ in range(n):
                    while True:
                        if fcur[0] is None:
                            if not fq:
                                return
                            fcur[0] = fq.popleft()
                        try:
                            next(fcur[0])
                            fpend[0] = max(0, fpend[0] - 1)
                            break
                        except StopIteration:
                            fcur[0] = None

            def feed_all():
                feed(1 << 30)

            _pools = [ps_f, ps_o]
            _pool_rr = [0]

            def _next_pool():
                _pool_rr[0] ^= 1
                return _pools[_pool_rr[0]]

            def v_sub(j, pool=None):
                # v^T j-tile (128 t cols, all 8 heads): 8 matmuls + bias add
                tc_, ti = j // 4, j % 4
                p_v = (pool or _next_pool()).tile([128, 512], F32, tag="pf", name="p_v")
                for kc in range(NK):
                    nc.tensor.matmul(
                        p_v[:],
                        xt[tc_][kc][:, ti * 128 : (ti + 1) * 128],
                        wv_sb[kc][:],
                        start=(kc == 0),
                        stop=(kc == NK - 1),
                    )
                    if kc % 2 == 1 and kc < NK - 1:
                        yield
                nc.vector.tensor_tensor(
                    vaug[j][:].rearrange("p (h c) -> p h c", h=HL)[:, :, 0:64],
                    p_v[:].rearrange("p (h c) -> p h c", h=HL),
                    bv_sb[:].rearrange("p (h c) -> p h c", h=HL),
                    mybir.AluOpType.add,
                )

            def qk_sub(nn, tc_, pool=None):
                # q^T/k^T block nn, t-chunk tc_: 8 matmuls + bias add
                p_qk = (pool or _next_pool()).tile([128, 512], F32, tag="pf", name="p_qk")
                for kc in range(NK):
                    nc.tensor.matmul(
                        p_qk[:],
                        wqk_sb[kc][:, nn * 128 : (nn + 1) * 128],
                        xt[tc_][kc][:],
                        start=(kc == 0),
                        stop=(kc == NK - 1),
                    )
                    if kc % 2 == 1 and kc < NK - 1:
                        yield
                nc.vector.tensor_scalar_add(
                    qkT[nn][:, tc_ * 512 : (tc_ + 1) * 512],
                    p_qk[:],
                    bqk_sb[:, nn : nn + 1],
                )

            def proj_unit(c, co):
                # one output tile [128 co, 512 t]: 4 matmuls + bias + DMA out
                p_o = _next_pool().tile([128, 512], F32, tag="pf", name="p_o")
                for hh in range(4):
                    nc.tensor.matmul(
                        p_o[:],
                        wproj_sb[hh][:, co * 128 : (co + 1) * 128],
                        yT_sb[hh][:, c * 512 : (c + 1) * 512],
                        start=(hh == 0),
                        stop=(hh == 3),
                    )
                    if hh == 1:
                        yield
                o_sb = ots.tile([128, 512], F32, tag="o")
                nc.vector.tensor_scalar_add(o_sb[:], p_o[:], bproj_sb[:, co : co + 1])
                nc.sync.dma_start(
                    outT_d[co * 128 : (co + 1) * 128, c * 512 : (c + 1) * 512],
                    o_sb[:],
                )

            def proj_chunk_gen(c):
                for co in range(8):
                    yield from proj_unit(c, co)

            def attention_chunk(h, c):
                poff = (h % 2) * 64
                kt = qkT[4 + h // 2]
                qt = qkT[h // 2]
                q0 = c * 512

                def va(j):
                    return vaug[j][:, h * 65 : (h + 1) * 65]

                # before issuing, spread this chunk's feeder backlog over its
                # pair slots so filler drains evenly
                n_pairs = 2 * c + 2
                quota = max(1, -(-fpend[0] * 2 // max(1, n_pairs * 3)))

                p_y = ps_y.tile([65, 512], F32, tag="py")

                qk_issues = []
                pv_issues = []

                def mk_offdiag(j0):
                    pt = [None]

                    def qk_i():
                        p_s = pp.tile([128, 1024], F32, tag="pp")
                        for half in range(2):
                            j = j0 + half
                            nc.tensor.matmul(
                                p_s[:, half * 512 : half * 512 + 512],
                                kt[poff : poff + 64, j * 128 : (j + 1) * 128],
                                qt[poff : poff + 64, q0 : q0 + 512],
                                start=True,
                                stop=True,
                            )
                        pt[0] = pts.tile([128, 1024], FP16, tag="pt", name="pt")
                        nc.scalar.activation(
                            pt[0][:], p_s[:], mybir.ActivationFunctionType.Exp
                        )

                    def pv_i():
                        for half in range(2):
                            j = j0 + half
                            nc.tensor.matmul(
                                p_y[:],
                                va(j),
                                pt[0][:, half * 512 : half * 512 + 512],
                                start=(j == 0),
                                stop=False,
                            )

                    qk_issues.append(qk_i)
                    pv_issues.append(pv_i)

                for j0 in range(0, 4 * c, 2):
                    mk_offdiag(j0)

                # diagonal block at 128-granularity (valid q-range only):
                # pair A: jd=0 (N=512 @cols 0:512), jd=1 (N=384 @cols 512:896)
                # pair B: jd=2 (N=256 @cols 0:256), jd=3 (N=128 @cols 256:384)
                ptA = [None]
                ptB = [None]
                tsel = [None] * 4

                def qk_diagA():
                    p_sA = pp.tile([128, 1024], F32, tag="pp")
                    nc.tensor.matmul(
                        p_sA[:, 0:512],
                        kt[poff : poff + 64, (4 * c) * 128 : (4 * c) * 128 + 128],
                        qt[poff : poff + 64, q0 : q0 + 512],
                        start=True,
                        stop=True,
                    )
                    nc.tensor.matmul(
                        p_sA[:, 512:896],
                        kt[
                            poff : poff + 64,
                            (4 * c + 1) * 128 : (4 * c + 1) * 128 + 128,
                        ],
                        qt[poff : poff + 64, q0 + 128 : q0 + 512],
                        start=True,
                        stop=True,
                    )
                    ptA[0] = pts.tile([128, 1024], FP16, tag="pt", name="ptA")
                    nc.scalar.activation(
                        ptA[0][:, 0:896],
                        p_sA[:, 0:896],
                        mybir.ActivationFunctionType.Exp,
                    )
                    for k, cols in ((0, (0, 128)), (1, (512, 640))):
                        t_t = tri.tile([128, 128], FP16, tag="tri")
                        nc.gpsimd.affine_select(
                            t_t[:],
                            ptA[0][:, cols[0] : cols[1]],
                            pattern=[[1, 128]],
                            compare_op=mybir.AluOpType.is_ge,
                            fill=0.0,
                            base=0,
                            channel_multiplier=-1,
                        )
                        tsel[k] = t_t

                def qk_diagB():
                    p_sB = pp.tile([128, 1024], F32, tag="pp")
                    # both key-tiles land in the same PSUM bank: start=True
                    # clears the whole bank, so only the first opens the group
                    nc.tensor.matmul(
                        p_sB[:, 0:256],
                        kt[
                            poff : poff + 64,
                            (4 * c + 2) * 128 : (4 * c + 2) * 128 + 128,
                        ],
                        qt[poff : poff + 64, q0 + 256 : q0 + 512],
                        start=True,
                        stop=False,
                    )
                    nc.tensor.matmul(
                        p_sB[:, 256:384],
                        kt[
                            poff : poff + 64,
                            (4 * c + 3) * 128 : (4 * c + 3) * 128 + 128,
                        ],
                        qt[poff : poff + 64, q0 + 384 : q0 + 512],
                        start=False,
                        stop=True,
                    )
                    ptB[0] = pts.tile([128, 1024], FP16, tag="pt", name="ptB")
                    nc.scalar.activation(
                        ptB[0][:, 0:384],
                        p_sB[:, 0:384],
                        mybir.ActivationFunctionType.Exp,
                    )
                    for k, cols in ((2, (0, 128)), (3, (256, 384))):
                        t_t = tri.tile([128, 128], FP16, tag="tri")
                        nc.gpsimd.affine_select(
                            t_t[:],
                            ptB[0][:, cols[0] : cols[1]],
                            pattern=[[1, 128]],
                            compare_op=mybir.AluOpType.is_ge,
                            fill=0.0,
                            base=0,
                            channel_multiplier=-1,
                        )
                        tsel[k] = t_t

                first = c == 0

                def pv_diagA():
                    # jd=0: tri q [0,128), rest q [128,512). start=True clears
                    # the whole p_y bank - only the first matmul may set it
                    nc.tensor.matmul(
                        p_y[:, 0:128], va(4 * c), tsel[0][:], start=first, stop=False
                    )
                    nc.tensor.matmul(
                        p_y[:, 128:512], va(4 * c), ptA[0][:, 128:512],
                        start=False, stop=False,
                    )
                    # jd=1: tri q [128,256), rest q [256,512)
                    nc.tensor.matmul(
                        p_y[:, 128:256], va(4 * c + 1), tsel[1][:],
                        start=False, stop=False,
                    )
                    nc.tensor.matmul(
                        p_y[:, 256:512], va(4 * c + 1), ptA[0][:, 640:896],
                        start=False, stop=False,
                    )

                def pv_diagB():
                    # jd=2: tri q [256,384), rest q [384,512)
                    nc.tensor.matmul(
                        p_y[:, 256:384], va(4 * c + 2), tsel[2][:],
                        start=False, stop=False,
                    )
                    nc.tensor.matmul(
                        p_y[:, 384:512], va(4 * c + 2), ptB[0][:, 128:256],
                        start=False, stop=False,
                    )
                    # jd=3: tri q [384,512)
                    nc.tensor.matmul(
                        p_y[:, 384:512], va(4 * c + 3), tsel[3][:],
                        start=False, stop=True,
                    )

                qk_issues.append(qk_diagA)
                pv_issues.append(pv_diagA)
                qk_issues.append(qk_diagB)
                pv_issues.append(pv_diagB)

                # woven issue: qk0, qk1, [qk_{k+2}, feed, pv_k] ... with the
                # previous chunk's deferred normalize slotted in after the
                # first QK pairs (its reciprocal has long finished by then)
                n = len(qk_issues)
                qk_issues[0]()
                if n > 1:
                    qk_issues[1]()
                fin = finish_pending.pop(0) if finish_pending else None
                for k in range(n):
                    if k + 2 < n:
                        qk_issues[k + 2]()
                    feed(quota)
                    if k == 0 and fin is not None:
                        fin()
                    pv_issues[k]()

                # normalize: y[d, t] = y_aug[d, t] / y_aug[64, t]
                # reciprocal issues now; the PE rank-1 broadcast, SBUF stage
                # and DVE multiply are deferred into the next chunk's stream
                r_sb = rts.tile([128, 512], F32R, tag="r")
                with nc.allow_low_precision(
                    reason="f32r holds full fp32-rounded reciprocal"
                ):
                    nc.vector.reciprocal(r_sb[64:65, :], p_y[64:65, :])

                def finish():
                    p_r = pp.tile([64, 512], F32, tag="pp")
                    nc.tensor.matmul(
                        p_r[:],
                        ones_sb[64:65, :],
                        r_sb[64:65, :],
                        start=True,
                        stop=True,
                    )
                    # DVE may read only one PSUM operand: stage the broadcast
                    rb_sb = rts.tile([64, 512], F32, tag="rb")
                    nc.vector.tensor_copy(rb_sb[:], p_r[:])
                    nc.vector.tensor_tensor(
                        yT_sb[h // 2][poff : poff + 64, q0 : q0 + 512],
                        p_y[0:64, :],
                        rb_sb[:],
                        mybir.AluOpType.mult,
                    )

                finish_pending.append(finish)

            # ---- static schedule ----
            # lead: v(tc0) + first qk blocks issued directly (back-to-back)
            for j in range(4):
                for _ in v_sub(j):
                    pass
            for _ in qk_sub(0, 0):
                pass
            for _ in qk_sub(4, 0):
                pass

            # rounds cover head pairs x c in {0,1,2}; c=3 + proj form the
            # tail. Each chunk's feeder blocks are enqueued just before it:
            # qk blocks arrive one chunk ahead of first use, v one chunk
            # ahead of the c-range that needs it.
            FEEDS = {
                (0, 0): [(qk_sub(0, 1), 4), (v_sub(4), 4), (v_sub(5), 4)],
                (1, 0): [(qk_sub(4, 1), 4), (v_sub(6), 4), (v_sub(7), 4)],
                (0, 1): [(qk_sub(0, 2), 4), (v_sub(8), 4), (v_sub(9), 4)],
                (1, 1): [(qk_sub(4, 2), 4), (v_sub(10), 4), (v_sub(11), 4)],
                (0, 2): [(qk_sub(1, 0), 4)],
                (1, 2): [(qk_sub(5, 0), 4)],
                (2, 0): [(qk_sub(1, 1), 4), (v_sub(12), 4), (v_sub(13), 4)],
                (3, 0): [(qk_sub(5, 1), 4), (v_sub(14), 4), (v_sub(15), 4)],
                (2, 1): [(qk_sub(1, 2), 4)],
                (3, 1): [(qk_sub(5, 2), 4)],
                (2, 2): [(qk_sub(2, 0), 4)],
                (3, 2): [(qk_sub(6, 0), 4)],
                (4, 0): [(qk_sub(2, 1), 4)],
                (5, 0): [(qk_sub(6, 1), 4)],
                (4, 1): [(qk_sub(2, 2), 4)],
                (5, 1): [(qk_sub(6, 2), 4)],
                (4, 2): [(qk_sub(3, 0), 4)],
                (5, 2): [(qk_sub(7, 0), 4)],
                (6, 0): [(qk_sub(3, 1), 4)],
                (7, 0): [(qk_sub(7, 1), 4)],
                (6, 1): [(qk_sub(3, 2), 4)],
                (7, 1): [(qk_sub(7, 2), 4)],
                (6, 2): [(qk_sub(0, 3), 4)],
                (7, 2): [(qk_sub(4, 3), 4)],
                (0, 3): [(qk_sub(1, 3), 4)],
                (1, 3): [(qk_sub(5, 3), 4)],
                (2, 3): [(qk_sub(2, 3), 4), (proj_chunk_gen(0), 16)],
                (3, 3): [(qk_sub(6, 3), 4)],
                (4, 3): [(qk_sub(3, 3), 4), (proj_chunk_gen(1), 16)],
                (5, 3): [(qk_sub(7, 3), 4)],
                (6, 3): [(proj_chunk_gen(2), 16)],
            }

            def run_chunk(h, c):
                for g, u in FEEDS.get((h, c), []):
                    feeder_add(g, u)
                attention_chunk(h, c)

            for i in range(4):
                for c in range(3):
                    run_chunk(2 * i, c)
                    run_chunk(2 * i + 1, c)
            for h in range(8):
                run_chunk(h, 3)
            feeder_add(proj_chunk_gen(3), 16)
            feed(1)
            for fin in finish_pending:
                fin()
            finish_pending.clear()
            feed_all()

    nc.compile()
    return nc


# ---------------------------------------------------------------------------
# host side
# ---------------------------------------------------------------------------

_CACHE = {}


def _get_runner():
    if "runner" in _CACHE:
        return _CACHE["runner"]

    import jax
    from jax.experimental.shard_map import shard_map
    from jax.sharding import Mesh, PartitionSpec

    from concourse.bass2jax import (
        _bass_exec_p,
        install_neuronx_cc_hook,
        partition_id_tensor,
    )

    install_neuronx_cc_hook()
    nc = build_nc()
    n_cores = 8

    partition_name = nc.partition_id_tensor.name if nc.partition_id_tensor else None
    in_names = []
    out_names = []
    out_avals = []
    for alloc in nc.m.functions[0].allocations:
        if not isinstance(alloc, mybir.MemoryLocationSet):
            continue
        name = alloc.memorylocations[0].name
        if alloc.kind == "ExternalInput":
            if name != partition_name:
                in_names.append(name)
        elif alloc.kind == "ExternalOutput":
            out_names.append(name)
            out_avals.append(
                jax.core.ShapedArray(tuple(alloc.tensor_shape), mybir.dt.np(alloc.dtype))
            )
    n_params = len(in_names)
    all_names = in_names + out_names
    if partition_name is not None:
        all_names = all_names + [partition_name]

    def _body(*args):
        operands = list(args)
        if partition_name is not None:
            operands.append(partition_id_tensor())
        outs = _bass_exec_p.bind(
            *operands,
            out_avals=tuple(out_avals),
            in_names=tuple(all_names),
            out_names=tuple(out_names),
            lowering_input_output_aliases=(),
            sim_require_finite=True,
            sim_require_nnan=True,
            nc=nc,
        )
        return tuple(outs)

    devices = jax.devices()[:n_cores]
    mesh = Mesh(np.asarray(devices), ("core",))
    n_outs = len(out_names)
    fn = jax.jit(
        shard_map(
            _body,
            mesh=mesh,
            in_specs=(PartitionSpec("core"),) * (n_params + n_outs),
            out_specs=(PartitionSpec("core"),) * n_outs,
            check_rep=False,
        ),
        keep_unused=True,
    )

    runner = {
        "fn": fn,
        "in_names": in_names,
        "out_names": out_names,
        "out_avals": out_avals,
        "n_cores": n_cores,
        "jax": jax,
    }
    _CACHE["runner"] = runner
    return runner


def _prepare_in_maps(x, w_attn, b_attn, w_proj, b_proj):
    x = np.asarray(x, dtype=np.float32)
    w_attn = np.asarray(w_attn, dtype=np.float32)
    b_attn = np.asarray(b_attn, dtype=np.float32)
    w_proj = np.asarray(w_proj, dtype=np.float32)
    b_proj = np.asarray(b_proj, dtype=np.float32)

    in_maps = []
    for core in range(8):
        b = core // 2
        h0 = HL * (core % 2)
        c0 = h0 * HD  # 512*(core%2)

        xT = np.ascontiguousarray(x[b].T).astype(np.float16)

        w_q = (w_attn[:, c0 : c0 + CL] * 0.125).astype(np.float16)
        w_k = w_attn[:, C + c0 : C + c0 + CL].astype(np.float16)
        wqk = np.ascontiguousarray(np.concatenate([w_q, w_k], axis=1))
        wv = np.ascontiguousarray(w_attn[:, 2 * C + c0 : 2 * C + c0 + CL]).astype(
            np.float16
        )
        wproj = np.ascontiguousarray(
            w_proj[c0 : c0 + CL, :].reshape(4, 128, C)
        ).astype(np.float16)

        b_q = b_attn[c0 : c0 + CL] * 0.125
        b_k = b_attn[C + c0 : C + c0 + CL]
        bqk = np.concatenate([b_q, b_k]).reshape(8, 128).T.astype(np.float32)
        bqk = np.ascontiguousarray(bqk)
        b_v = b_attn[2 * C + c0 : 2 * C + c0 + CL].astype(np.float32)
        bv = np.ascontiguousarray(np.broadcast_to(b_v[None, :], (128, CL)))
        if core % 2 == 0:
            bp = np.ascontiguousarray(b_proj.reshape(8, 128).T.astype(np.float32))
        else:
            bp = np.zeros((128, 8), dtype=np.float32)

        in_maps.append(
            {
                "xT": xT,
                "wqk": wqk,
                "wv": wv,
                "wproj": wproj,
                "bqk": bqk,
                "bv": bv,
                "bproj": bp,
            }
        )
    return in_maps


def _run_device(in_maps):
    r = _get_runner()
    jax = r["jax"]
    n = r["n_cores"]
    per_core = [[np.asarray(m[name]) for name in r["in_names"]] for m in in_maps]
    concat_in = [
        np.concatenate([per_core[c][i] for c in range(n)], axis=0)
        for i in range(len(r["in_names"]))
    ]
    concat_zero = [
        np.zeros((n * a.shape[0], *a.shape[1:]), a.dtype) for a in r["out_avals"]
    ]
    outs = r["fn"](*[jax.device_put(a) for a in concat_in + concat_zero])
    jax.block_until_ready(outs)
    (outT,) = [np.asarray(o) for o in outs]
    return outT.reshape(n, C, T)


def kernel(x, w_attn, b_attn, w_proj, b_proj):
    in_maps = _prepare_in_maps(x, w_attn, b_attn, w_proj, b_proj)
    outT = _run_device(in_maps)
    # host gather: sum the two head-halves of each batch, transpose back
    out = np.empty((B, T, C), dtype=np.float32)
    for b in range(B):
        out[b] = (outT[2 * b] + outT[2 * b + 1]).T
    return out
